# revision 1
# baseline (speedup 1.0000x reference)
"""Multi-head attention (B=2, S=2048, D=1024, H=16) on 8 Trainium2 NeuronCores.

Sharding (per the batch+head hint): core c handles batch b=c//4 and head-group
g=c%4 (4 heads, i.e. a 256-column slice of the QKV projections and a 256-row
slice of Wo).  Each core computes q^T/k^T/v projections for its head group,
attention in transposed score space (scores^T = k^T-tile.T @ q^T, softmax
denominator via a ones-augmented V column in the PV matmul), and its
out-projection partial  ctx_g @ Wo[256g:256(g+1), :].

The out_proj reduction over the 4 head-group cores of each batch is done on
the host (device collectives on this stack cost ~145us for 8MB - far more
than the arithmetic they replace - so the partial-sum gather IS the unshard
step).  Biases: bq/bk are applied on device (they feed the softmax
nonlinearly); bv/bo commute through attention/out_proj linearly and are folded
into a single host-side correction vector  c = bv @ Wo + bo.

Inputs are pre-transposed on the host (x^T layout) because TensorE consumes
the contraction dimension on partitions.  All matmul operands are bf16
(fp32 PSUM accumulation): rel-err ~5e-3 passes the 2e-2 gate with margin,
and bf16 gets fast-weight-load that pipelines behind the previous matmul.

Schedule notes (from NTFF traces):
- The scores matmuls have K=64 (head dim): the head pair sits at SBUF
  partitions 0-63 / 64-127, which bass auto-lowers to PE row-tiles (0,0) and
  (64,0); the HW runs each pair concurrently (~225ns for both).
- The attention inner loop is Scalar-engine bound (exp of 16.8M scores/core
  ~ 142us); all other engine work is scheduled to hide inside it:
  q-projection halves and the deferred out-projection of the previous chunk
  fill TensorE between score/PV bursts.
- Softmax normalize: DVE reciprocal of the PSUM denominator row, a K=1 PE
  matmul broadcasts it across 64 partitions, DVE multiply -> ctxT.  (The
  previous DRAM-bounce broadcast stalled TensorE ~11us per chunk.)
- exp() activation table is preloaded at kernel start so the ~2.7us
  ACT_TABLE_LOAD overlaps the k/v projection phase.
- x^T inputs live in SBUF whole (12MB bf16 across 3 tensors), DMA'd in
  per-k-tile slices with 4KB-contiguous lines.
"""

import numpy as np
import ml_dtypes

import concourse.bass as bass
import concourse.mybir as mybir
import concourse.tile as tile
from concourse import bacc
from concourse.bass_utils import run_bass_kernel_spmd

B, S, D, H = 2, 2048, 1024, 16
HD = D // H          # 64 head dim
NCORE = 8
G = NCORE // B       # 4 head-groups per batch
HG = H // G          # 4 heads per group
DG = D // G          # 256 projection columns per group
P = 128              # partitions
KT = D // P          # 8 contraction tiles for projections
CH = 512             # s-chunk (projection rhs width & attention sq chunk)
NJ = S // CH         # 4 chunks
STILES = S // P      # 16 sk tiles
TB = 2               # scores t-batch per exp op (psum: [128, TB*512] = 2 banks)
VBLK = HD + 1        # v block: 64 v cols + 1 ones col (softmax denominator)

f32 = mybir.dt.float32
bf16 = mybir.dt.bfloat16
MM_DT = mybir.dt.bfloat16
NP_MM = np.float32 if MM_DT == mybir.dt.float32r else ml_dtypes.bfloat16
EXP = mybir.ActivationFunctionType.Exp
SCALE = 1.0 / np.sqrt(np.float32(HD))


def _build_program():
    nc = bacc.Bacc("TRN2", target_bir_lowering=False, debug=False,
                   num_devices=NCORE)

    # All inputs arrive pre-tiled on the host into the exact SBUF layouts
    # (partition-major, chunk-major for x) so every DMA moves 4-8KB
    # contiguous lines per partition at full HBM bandwidth.
    xqT_d = nc.dram_tensor("xqT", [P, NJ * KT * CH], MM_DT,
                           kind="ExternalInput")
    xkT_d = nc.dram_tensor("xkT", [P, NJ * KT * CH], MM_DT,
                           kind="ExternalInput")
    xvT_d = nc.dram_tensor("xvT", [P, NJ * KT * CH], MM_DT,
                           kind="ExternalInput")
    wq_d = nc.dram_tensor("wq", [P, KT * DG], MM_DT, kind="ExternalInput")
    wk_d = nc.dram_tensor("wk", [P, KT * DG], MM_DT, kind="ExternalInput")
    wv_d = nc.dram_tensor("wv", [P, KT * DG], MM_DT, kind="ExternalInput")
    wo_d = nc.dram_tensor("wo", [P, 2 * D], MM_DT, kind="ExternalInput")
    bq_d = nc.dram_tensor("bqk", [P, 4], f32, kind="ExternalInput")
    out_d = nc.dram_tensor("out", [S, D], MM_DT, kind="ExternalOutput")

    with tile.TileContext(nc) as tc:
        _emit(nc, tc, xqT_d, xkT_d, xvT_d, wq_d, wk_d, wv_d, wo_d, bq_d, out_d)
    nc.compile()
    return nc


def _emit(nc, tc, xqT_d, xkT_d, xvT_d, wq_d, wk_d, wv_d, wo_d, bq_d, out_d):
    from contextlib import ExitStack
    ctx = ExitStack()
    with ctx:
        consts = ctx.enter_context(tc.tile_pool(name="consts", bufs=1))
        persist = ctx.enter_context(tc.tile_pool(name="persist", bufs=1))
        xfull = ctx.enter_context(tc.tile_pool(name="xfull", bufs=1))
        epool = ctx.enter_context(tc.tile_pool(name="exps", bufs=4))
        small = ctx.enter_context(tc.tile_pool(name="small", bufs=4))
        bpool = ctx.enter_context(tc.tile_pool(name="bcast", bufs=4))
        opool = ctx.enter_context(tc.tile_pool(name="ostage", bufs=3))
        ps_s = ctx.enter_context(tc.tile_pool(name="ps_s", bufs=2, space="PSUM"))
        ps_acc = ctx.enter_context(tc.tile_pool(name="ps_acc", bufs=2, space="PSUM"))
        ps_pr = ctx.enter_context(tc.tile_pool(name="ps_pr", bufs=2, space="PSUM"))

        ones1_d = nc.inline_tensor(np.ones((1, HD), NP_MM), name="ones_row")
        ones1_ap = ones1_d.ap()
        if ones1_ap.dtype != MM_DT:
            ones1_ap = ones1_ap.bitcast(MM_DT)

        # ---- weights + full x^T staging.  x arrives in per-s-chunk DMAs,
        # k/v/q interleaved, so kv-projection chunk c and attention chunk 0
        # unblock after ~3c+3 DMAs instead of after the full 12MB ----------
        def load_w(w_d, tag):
            t = consts.tile([P, KT * DG], MM_DT, tag=tag)
            nc.sync.dma_start(out=t[:], in_=w_d.ap())
            return t

        def make_x(tag):
            # chunk-major SBUF layout: [p, c, kt, s'] so a chunk is one
            # contiguous 8KB run per partition
            t = xfull.tile([P, NJ * KT * CH], MM_DT, tag=tag, name=tag)
            return t.rearrange("p (c kt s) -> p c kt s", c=NJ, kt=KT)

        xk, xv, xq = make_x("xk"), make_x("xv"), make_x("xq")

        def load_x_chunk(tv, x_d, c):
            nc.sync.dma_start(
                out=tv[:, c],
                in_=x_d.rearrange("p (c kt s) -> p c kt s",
                                  c=NJ, kt=KT)[:, c])

        # issue order: wk + xk chunk 0 (split in k-halves so the first
        # kproj matmuls start on the first half-MB) ahead of everything,
        # then each weight right before the first x chunk needing it; wo
        # (needed last) at the end behind everything else.
        wk_sb = load_w(wk_d, "wk")
        xk_src = xkT_d.rearrange("p (c kt s) -> p c kt s", c=NJ, kt=KT)
        nc.sync.dma_start(out=xk[:, 0, 0:KT // 2], in_=xk_src[:, 0, 0:KT // 2])
        nc.sync.dma_start(out=xk[:, 0, KT // 2:], in_=xk_src[:, 0, KT // 2:])

        bqk_sb = consts.tile([P, 4], f32, tag="bqk")  # [bq|bk] x m-half
        nc.sync.dma_start(out=bqk_sb[:], in_=bq_d.ap())
        ones1_sb = consts.tile([1, HD], MM_DT, tag="ones1")
        nc.sync.dma_start(out=ones1_sb[:], in_=ones1_ap)
        # preload the exp activation-table set (~2.7us) behind startup DMA
        dume = small.tile([1, 4], f32, tag="dume")
        nc.scalar.activation(dume[:], ones1_sb[0:1, 0:4], EXP)

        wv_sb = load_w(wv_d, "wv")
        load_x_chunk(xv, xvT_d, 0)
        wq_sb = load_w(wq_d, "wq")
        load_x_chunk(xq, xqT_d, 0)
        for c in range(1, NJ):
            for tv, x_d in ((xk, xkT_d), (xv, xvT_d), (xq, xqT_d)):
                load_x_chunk(tv, x_d, c)
        wo_sb = consts.tile([P, 2 * D], MM_DT, tag="wo")  # 2 k-tiles [128, D]
        nc.sync.dma_start(out=wo_sb[:], in_=wo_d.ap())

        # persistent activations
        qT = [persist.tile([P, S], MM_DT, tag=f"qT{m}", name=f"qT{m}")
              for m in range(2)]
        kT = [persist.tile([P, S], MM_DT, tag=f"kT{m}", name=f"kT{m}")
              for m in range(2)]
        v_sb = persist.tile([P, HG * STILES * VBLK], MM_DT, tag="v")
        ctxT = [persist.tile([P, S], MM_DT, tag=f"ctxT{m}", name=f"ctxT{m}")
                for m in range(2)]
        v_view = v_sb.rearrange("p (h t c) -> p h t c", h=HG, t=STILES)

        # ones columns for the softmax-denominator rows of the PV matmuls
        # (memset instead of a 64-elements/partition scatter DMA)
        nc.gpsimd.memset(v_view[:, :, :, HD], 1.0)

        def proj_qk_half(xc, w_sb, dst, bias_i, j, m):
            # dst[m][dq, j*CH:+CH] = (W[:, m-half].T @ x^T-chunk) + bias
            acc = ps_pr.tile([P, CH], f32, tag="pr", name="pr")
            for k in range(KT):
                nc.tensor.matmul(
                    acc[:], w_sb[:, k * DG + m * P:k * DG + (m + 1) * P],
                    xc[:, j, k, :],
                    start=(k == 0), stop=(k == KT - 1))
            nc.vector.tensor_add(
                dst[m][:, j * CH:(j + 1) * CH], acc[:],
                bqk_sb[:, 2 * bias_i + m:2 * bias_i + m + 1].broadcast_to(
                    [P, CH]))

        def proj_v(j):
            # v rows j*CH..: 4 s-subtiles of 128; heads land in v_view blocks
            for si in range(CH // P):
                st = j * (CH // P) + si
                acc = ps_pr.tile([P, DG], f32, tag="pr", name="pr")
                for k in range(KT):
                    nc.tensor.matmul(
                        acc[:], xv[:, j, k, si * P:(si + 1) * P],
                        wv_sb[:, k * DG:(k + 1) * DG],
                        start=(k == 0), stop=(k == KT - 1))
                nc.vector.tensor_copy(
                    v_view[:, :, st, 0:HD],
                    acc[:].rearrange("p (h c) -> p h c", h=HG))

        def scores_step(j, m, tb):
            jc = slice(j * CH, (j + 1) * CH)
            sps = [ps_s.tile([P, TB * CH], f32, tag="s", name=f"s{i}")
                   for i in range(2)]
            for tt in range(TB):
                t = tb * TB + tt
                for hh in range(2):  # row-tile pair -> concurrent on PE
                    lo, hi = hh * HD, (hh + 1) * HD
                    nc.tensor.matmul(
                        sps[hh][:, tt * CH:(tt + 1) * CH],
                        kT[m][lo:hi, t * P:(t + 1) * P],
                        qT[m][lo:hi, jc], start=True, stop=True)
            return sps

        def exp_step(sps):
            es = []
            for hh in range(2):
                e = epool.tile([P, TB * CH], MM_DT, tag="e", name="e")
                nc.scalar.activation(e[:], sps[hh][:], EXP, scale=SCALE)
                es.append(e)
            return es

        def pv_step(accs, m, tb, es):
            for tt in range(TB):
                t = tb * TB + tt
                for hh in range(2):
                    h = 2 * m + hh
                    nc.tensor.matmul(
                        accs[hh][:], v_view[:, h, t, :],
                        es[hh][:, tt * CH:(tt + 1) * CH],
                        start=(t == 0), stop=(t == STILES - 1))

        def stage_accs(accs):
            # Copy psum ctx+denom to SBUF (bf16) right after the PV stop so
            # the psum banks free in ~0.8us; the normalize chain then runs
            # entirely off the critical path from these copies.
            stg = []
            with nc.allow_low_precision(
                    reason="bf16 softmax ctx/denom: ~0.4% of a 2e-2 budget"):
                for hh in range(2):
                    s = bpool.tile([VBLK, CH], MM_DT, tag=f"stg{hh}",
                                   name="stg")
                    nc.vector.tensor_copy(s[:], accs[hh][:])
                    stg.append(s)
            return stg

        def recip_chain(stg):
            # 1/denom on 128 DVE lanes: reshape the [1,512] denominator rows
            # to [128,4] via sbuf-sbuf DMA (a 1-lane DVE reciprocal costs
            # ~3.3us; 128-lane costs ~70ns), then reshape back for the PE
            # broadcast rhs.
            # reshape DMAs ride the idle GpSimd (SWDGE) queue: keeps them
            # off the SP queue's FIFO (out-DMAs) and shortens chain latency
            dq = small.tile([P, 8], MM_DT, tag="dq", name="dq")
            for hh in range(2):
                nc.gpsimd.dma_start(
                    dq[:, 4 * hh:4 * hh + 4],
                    stg[hh][HD:HD + 1, :].rearrange("o (p c) -> o p c", p=P))
            rq = small.tile([P, 8], MM_DT, tag="rq", name="rq")
            with nc.allow_low_precision(
                    reason="bf16 softmax recip: ~0.4% of a 2e-2 budget"):
                nc.vector.reciprocal(rq[:], dq[:])
            rrs = []
            for hh in range(2):
                rr = bpool.tile([1, CH], MM_DT, tag=f"rr{hh}", name="rr")
                nc.gpsimd.dma_start(
                    rr[:].rearrange("o (p c) -> o p c", p=P),
                    rq[:, 4 * hh:4 * hh + 4])
                rrs.append(rr)
            return rrs

        def apply_norm(j, m, stg, rrs):
            # ctxT[m][:, jc] = stg[0:HD] * broadcast(rr); the broadcast is
            # a K=1 PE matmul: ones[1,HD].T @ rr[1,CH] -> psum [HD, CH].
            jc = slice(j * CH, (j + 1) * CH)
            for hh in range(2):
                bb = ps_pr.tile([P, CH], f32, tag="pr", name="bb")
                nc.tensor.matmul(bb[0:HD, :], ones1_sb[:], rrs[hh][:],
                                 start=True, stop=True)
                bsb = bpool.tile([HD, CH], MM_DT, tag="bsb", name="bsb")
                nc.vector.tensor_copy(bsb[:], bb[0:HD, :])
                if hh == 0:
                    nc.vector.tensor_mul(ctxT[m][0:HD, jc],
                                         stg[hh][0:HD, :], bsb[:])
                else:
                    tmp = bpool.tile([HD, CH], MM_DT, tag="tmp", name="tmp")
                    nc.vector.tensor_mul(tmp[:], stg[hh][0:HD, :], bsb[:])
                    # shift to partitions 64..127 (cross-partition: DMA)
                    nc.gpsimd.dma_start(ctxT[m][HD:P, jc], tmp[:])

        def outproj_item(j, si, nh):
            # one [128, 512] block of chunk j's out-projection
            st = j * (CH // P) + si
            po = ps_pr.tile([P, CH], f32, tag="pr", name="po")
            for m in range(2):
                nc.tensor.matmul(
                    po[:], ctxT[m][:, st * P:(st + 1) * P],
                    wo_sb[:, m * D + nh * CH:m * D + (nh + 1) * CH],
                    start=(m == 0), stop=(m == 1))
            ostage = opool.tile([P, CH], MM_DT, tag="ostage", name="ostage")
            nc.vector.tensor_copy(ostage[:], po[:])
            nc.sync.dma_start(
                out_d[st * P:(st + 1) * P, nh * CH:(nh + 1) * CH],
                ostage[:])

        # ---- schedule: one flat software-pipelined stream over the 64
        # (j, m, tb) steps.  Each step emits scores(s) then pv(s-1) then
        # exp(s), so ScalarE's exp stream never waits for a PV that itself
        # waits on exp output, and never breaks at m/j section boundaries.
        # Independent tensor work (kv/q projections, deferred normalize and
        # out-projection of completed chunks) rides per-step filler slots.
        sections = [(j, m) for j in range(NJ) for m in range(2)]
        steps = [(k, j, m, tb)
                 for k, (j, m) in enumerate(sections)
                 for tb in range(STILES // TB)]
        fillers = {}

        def add_filler(s, f):
            fillers.setdefault(s, []).append(f)

        # static fillers: q projections ahead of their section; kv chunks
        # 1-3 inside section 0 just before the t-tiles that need them
        def kv_filler(c):
            def f():
                for m in range(2):
                    proj_qk_half(xk, wk_sb, kT, 1, c, m)
            return f

        add_filler(1, kv_filler(1))
        add_filler(2, lambda: proj_v(1))
        add_filler(3, kv_filler(2))
        add_filler(4, lambda: proj_v(2))
        add_filler(5, kv_filler(3))
        add_filler(6, lambda: proj_v(3))
        add_filler(7, lambda: proj_qk_half(xq, wq_sb, qT, 0, 0, 1))
        # q projections go far ahead of their section: the bias-add rides a
        # deep DVE queue, so give it ~8 steps (17us) of slack
        for j in range(1, NJ):
            add_filler(16 * j - 8,
                       lambda j=j: proj_qk_half(xq, wq_sb, qT, 0, j, 0))
            add_filler(16 * j + 1,
                       lambda j=j: proj_qk_half(xq, wq_sb, qT, 0, j, 1))

        def finish_section(k, s_now):
            # section k's accs are complete: stage out of psum, start the
            # reciprocal chain, and schedule its normalize + (for m=1) the
            # chunk's out-projection a few steps out.
            j, m = sections[k]
            stg = stage_accs(acc_state.pop(k))
            rrs = recip_chain(stg)
            if s_now is None:
                apply_norm(j, m, stg, rrs)
                return
            # +3 steps (~6us) so the bb matmul never waits the reshape-DMA
            # chain latency inside the tensor FIFO
            add_filler(s_now + 3,
                       lambda: apply_norm(j, m, stg, rrs))
            if m == 1:
                for i in range(8):
                    si, nh = divmod(i, 2)
                    add_filler(s_now + 4 + i,
                               lambda si=si, nh=nh: outproj_item(j, si, nh))

        # prologue: kv/q projections for chunk 0
        for m in range(2):
            proj_qk_half(xk, wk_sb, kT, 1, 0, m)
        proj_v(0)
        proj_qk_half(xq, wq_sb, qT, 0, 0, 0)

        acc_state = {}
        prev = None      # (k, m, tb, es) of the previous step
        for s, (k, j, m, tb) in enumerate(steps):
            for f in fillers.pop(s, ()):
                f()
            sps = scores_step(j, m, tb)
            if prev is not None:
                pk, pm, ptb, pes = prev
                if pk not in acc_state:
                    acc_state[pk] = [
                        ps_acc.tile([VBLK, CH], f32, tag="acc", name=f"acc{i}")
                        for i in range(2)]
                pv_step(acc_state[pk], pm, ptb, pes)
                if ptb == STILES // TB - 1:
                    finish_section(pk, s)
            es = exp_step(sps)
            prev = (k, m, tb, es)
        pk, pm, ptb, pes = prev
        pv_step(acc_state[pk], pm, ptb, pes)
        finish_section(pk, None)
        for si in range(CH // P):
            for nh in range(2):
                outproj_item(NJ - 1, si, nh)
        for f in sorted(fillers):
            for g in fillers[f]:
                g()


_NC_CACHE = {}


def _get_program():
    if "nc" not in _NC_CACHE:
        _NC_CACHE["nc"] = _build_program()
    return _NC_CACHE["nc"]


def _make_in_maps(inputs):
    query = np.asarray(inputs["query"], dtype=np.float32)
    key = np.asarray(inputs["key"], dtype=np.float32)
    value = np.asarray(inputs["value"], dtype=np.float32)
    Wq = np.asarray(inputs["Wq"], dtype=np.float32)
    Wk = np.asarray(inputs["Wk"], dtype=np.float32)
    Wv = np.asarray(inputs["Wv"], dtype=np.float32)
    Wo = np.asarray(inputs["Wo"], dtype=np.float32)
    bq = np.asarray(inputs["bq"], dtype=np.float32)
    bk = np.asarray(inputs["bk"], dtype=np.float32)

    # pre-tile into the device SBUF layouts (partition-major; x chunk-major)
    def tile_x(xTb):  # [D, S] -> [P, NJ*KT*CH]
        return np.ascontiguousarray(
            xTb.reshape(KT, P, NJ, CH).transpose(1, 2, 0, 3).reshape(
                P, NJ * KT * CH))

    def tile_w(w):  # [D, cols] -> [P, KT*cols]
        n = w.shape[1]
        return np.ascontiguousarray(
            w.reshape(KT, P, n).transpose(1, 0, 2).reshape(P, KT * n))

    xT = {}
    for b in range(B):
        xT[("q", b)] = tile_x(query[b].T.astype(NP_MM))
        xT[("k", b)] = tile_x(key[b].T.astype(NP_MM))
        xT[("v", b)] = tile_x(value[b].T.astype(NP_MM))

    in_maps = []
    for c in range(NCORE):
        b, g = divmod(c, G)
        cols = slice(g * DG, (g + 1) * DG)
        wo_g = Wo[cols, :].astype(NP_MM)  # [DG, D] = [(2 kt, P), D]
        in_maps.append({
            "xqT": xT[("q", b)],
            "xkT": xT[("k", b)],
            "xvT": xT[("v", b)],
            "wq": tile_w(Wq[:, cols].astype(NP_MM)),
            "wk": tile_w(Wk[:, cols].astype(NP_MM)),
            "wv": tile_w(Wv[:, cols].astype(NP_MM)),
            "wo": np.ascontiguousarray(
                wo_g.reshape(2, P, D).transpose(1, 0, 2).reshape(P, 2 * D)),
            "bqk": np.ascontiguousarray(
                np.stack([bq[cols], bk[cols]]).reshape(2, 2, P).transpose(
                    2, 0, 1).reshape(P, 4)),
        })
    return in_maps


def kernel(query, key, value, Wq, bq, Wk, bk, Wv, bv, Wo, bo):
    bv = np.asarray(bv, dtype=np.float32)
    bo = np.asarray(bo, dtype=np.float32)
    Wo = np.asarray(Wo, dtype=np.float32)

    nc = _get_program()
    in_maps = _make_in_maps({
        "query": query, "key": key, "value": value, "Wq": Wq, "Wk": Wk,
        "Wv": Wv, "Wo": Wo, "bq": bq, "bk": bk,
    })

    res = run_bass_kernel_spmd(nc, in_maps, list(range(NCORE)))

    # unshard: sum the 4 head-group partials per batch (bf16 -> f32); add
    # the linear bias correction (bv and bo commute through attn/out_proj).
    corr = bv @ Wo + bo
    out = np.empty((B, S, D), dtype=np.float32)
    for b in range(B):
        acc = res.results[4 * b]["out"].astype(np.float32)
        for g in range(1, G):
            acc += res.results[4 * b + g]["out"].astype(np.float32)
        out[b] = acc + corr
    return out



# revision 6
# speedup vs baseline: 1.0038x; 1.0038x over previous
"""Multi-head attention (B=2, S=2048, D=1024, H=16) on 8 Trainium2 NeuronCores.

Sharding (per the batch+head hint): core c handles batch b=c//4 and head-group
g=c%4 (4 heads, i.e. a 256-column slice of the QKV projections and a 256-row
slice of Wo).  Each core computes q^T/k^T/v projections for its head group,
attention in transposed score space (scores^T = k^T-tile.T @ q^T, softmax
denominator via a ones-augmented V column in the PV matmul), and its
out-projection partial  ctx_g @ Wo[256g:256(g+1), :].

The out_proj reduction over the 4 head-group cores of each batch is done on
the host (device collectives on this stack cost ~145us for 8MB - far more
than the arithmetic they replace - so the partial-sum gather IS the unshard
step).  Biases: bq/bk are applied on device (they feed the softmax
nonlinearly); bv/bo commute through attention/out_proj linearly and are folded
into a single host-side correction vector  c = bv @ Wo + bo.

Inputs are pre-transposed on the host (x^T layout) because TensorE consumes
the contraction dimension on partitions.  All matmul operands are bf16
(fp32 PSUM accumulation): rel-err ~5e-3 passes the 2e-2 gate with margin.
fp8 was evaluated and rejected: relative error through a matmul chain stays
at the per-element quantization level (~4-6%), far beyond the gate.

Schedule notes (from NTFF traces):
- Step = (chunk j, head-pair m, k-tile t).  Both heads' score matmuls of a
  step write ONE [128, 1024] psum tile (h0 -> cols 0:512, h1 -> 512:1024) and
  ONE exp op consumes it.  This makes the score pair's WAR hazard (against
  exp of step s-2) atomic, so both MMs become ready together, stay adjacent
  in the tensor queue, and the PE row-tile packer runs them concurrently
  (lhsT base partitions 0/64 -> tile_position (0,0)/(64,0), ~220ns/pair
  instead of 2x216).  The old per-head psum tiles made h1's readiness lag
  h0's by a full exp (staggered exp deps), breaking the pairing ~half the
  time.
- The attention stream is Scalar-bound in theory (exp of 16.8M scores/core
  ~147us at (N+352)/1.2 ns per [128,1024] ACT op) but PE-bound in practice
  (~165us): projections 28us + vproj 14us + scores 42us + PV 56us +
  outproj 14us + exposure.  All other engine work hides inside it.
- Softmax normalize: 1/denom on 128 DVE lanes (gpsimd-DMA reshape [1,512] ->
  [128,4], reciprocal, reshape back), then a GpSimd partition_broadcast of
  the [1,512] reciprocal row to 64 partitions and a DVE multiply per head.
  DVE ops tolerate mismatched in/out partition bases, so h1's product is
  written directly to ctxT partitions 64:127 (no DMA shift).  NOTE:
  partition_broadcast itself silently mis-broadcasts when its OUT base
  partition is 64 - always broadcast into a base-0 tile.
- The last section's normalize chain + out-projection are the kernel tail:
  it reads ctx/denom directly from PSUM (no staging - nothing needs the
  banks afterwards) and a handful of warm-up matmuls bridge the recip
  chain's ~4us PE idle so the tail outproj doesn't run at the HAM-throttled
  1.2GHz clock.
- exp() activation table is preloaded at kernel start so the ~2.7us
  ACT_TABLE_LOAD overlaps the k/v projection phase.
- x^T kv inputs are DMA'd upfront (needed within ~12us); xq chunks 1-3 and
  Wo are issued from inside the step stream so the startup DMAs that gate
  the first matmuls get the full HBM bandwidth.
"""

import numpy as np
import ml_dtypes

import concourse.bass as bass
import concourse.mybir as mybir
import concourse.tile as tile
from concourse import bacc
from concourse.bass_utils import run_bass_kernel_spmd

B, S, D, H = 2, 2048, 1024, 16
HD = D // H          # 64 head dim
NCORE = 8
G = NCORE // B       # 4 head-groups per batch
HG = H // G          # 4 heads per group
DG = D // G          # 256 projection columns per group
P = 128              # partitions
KT = D // P          # 8 contraction tiles for projections
CH = 512             # s-chunk (projection rhs width & attention sq chunk)
NJ = S // CH         # 4 chunks
STILES = S // P      # 16 sk tiles
VBLK = HD + 1        # v block: 64 v cols + 1 ones col (softmax denominator)
NSEC = NJ * 2        # 8 sections (j, m)
NSTEP = NSEC * STILES  # 128 pipeline steps

f32 = mybir.dt.float32
bf16 = mybir.dt.bfloat16
MM_DT = mybir.dt.bfloat16
NP_MM = np.float32 if MM_DT == mybir.dt.float32r else ml_dtypes.bfloat16
EXP = mybir.ActivationFunctionType.Exp
SCALE = 1.0 / np.sqrt(np.float32(HD))


def _build_program():
    nc = bacc.Bacc("TRN2", target_bir_lowering=False, debug=False,
                   num_devices=NCORE)

    # All inputs arrive pre-tiled on the host into the exact SBUF layouts
    # (partition-major, chunk-major for x) so every DMA moves 4-8KB
    # contiguous lines per partition at full HBM bandwidth.
    xqT_d = nc.dram_tensor("xqT", [P, NJ * KT * CH], MM_DT,
                           kind="ExternalInput")
    xkT_d = nc.dram_tensor("xkT", [P, NJ * KT * CH], MM_DT,
                           kind="ExternalInput")
    xvT_d = nc.dram_tensor("xvT", [P, NJ * KT * CH], MM_DT,
                           kind="ExternalInput")
    wq_d = nc.dram_tensor("wq", [P, KT * DG], MM_DT, kind="ExternalInput")
    wk_d = nc.dram_tensor("wk", [P, KT * DG], MM_DT, kind="ExternalInput")
    wv_d = nc.dram_tensor("wv", [P, KT * DG], MM_DT, kind="ExternalInput")
    wo_d = nc.dram_tensor("wo", [P, 2 * D], MM_DT, kind="ExternalInput")
    bq_d = nc.dram_tensor("bqk", [P, 4], f32, kind="ExternalInput")
    out_d = nc.dram_tensor("out", [S, D], MM_DT, kind="ExternalOutput")

    with tile.TileContext(nc) as tc:
        _emit(nc, tc, xqT_d, xkT_d, xvT_d, wq_d, wk_d, wv_d, wo_d, bq_d, out_d)
    nc.compile()
    return nc


def _emit(nc, tc, xqT_d, xkT_d, xvT_d, wq_d, wk_d, wv_d, wo_d, bq_d, out_d):
    from contextlib import ExitStack
    ctx = ExitStack()
    with ctx:
        consts = ctx.enter_context(tc.tile_pool(name="consts", bufs=1))
        persist = ctx.enter_context(tc.tile_pool(name="persist", bufs=1))
        xfull = ctx.enter_context(tc.tile_pool(name="xfull", bufs=1))
        epool = ctx.enter_context(tc.tile_pool(name="exps", bufs=4))
        small = ctx.enter_context(tc.tile_pool(name="small", bufs=4))
        bpool = ctx.enter_context(tc.tile_pool(name="bcast", bufs=4))
        opool = ctx.enter_context(tc.tile_pool(name="ostage", bufs=3))
        ps_s = ctx.enter_context(tc.tile_pool(name="ps_s", bufs=2, space="PSUM"))
        ps_acc = ctx.enter_context(tc.tile_pool(name="ps_acc", bufs=2, space="PSUM"))
        ps_pr = ctx.enter_context(tc.tile_pool(name="ps_pr", bufs=2, space="PSUM"))

        # ---- weights + x^T staging.  kv x chunks are needed within the
        # first ~12us (the attention k-sweep of section 0 spans all of S),
        # so they stream upfront; xq chunks 1-3 and wo are issued from
        # filler slots inside the step stream instead, so the startup DMAs
        # they'd otherwise compete with finish sooner. ------------------
        def load_w(w_d, tag):
            t = consts.tile([P, KT * DG], MM_DT, tag=tag)
            nc.sync.dma_start(out=t[:], in_=w_d.ap())
            return t

        def make_x(tag):
            # chunk-major SBUF layout: [p, c, kt, s'] so a chunk is one
            # contiguous 8KB run per partition
            t = xfull.tile([P, NJ * KT * CH], MM_DT, tag=tag, name=tag)
            return t.rearrange("p (c kt s) -> p c kt s", c=NJ, kt=KT)

        xk, xv, xq = make_x("xk"), make_x("xv"), make_x("xq")

        def load_x_chunk(tv, x_d, c):
            nc.sync.dma_start(
                out=tv[:, c],
                in_=x_d.rearrange("p (c kt s) -> p c kt s",
                                  c=NJ, kt=KT)[:, c])

        wk_sb = load_w(wk_d, "wk")
        xk_src = xkT_d.rearrange("p (c kt s) -> p c kt s", c=NJ, kt=KT)
        nc.sync.dma_start(out=xk[:, 0, 0:KT // 2], in_=xk_src[:, 0, 0:KT // 2])
        nc.sync.dma_start(out=xk[:, 0, KT // 2:], in_=xk_src[:, 0, KT // 2:])

        bqk_sb = consts.tile([P, 4], f32, tag="bqk")  # [bq|bk] x m-half
        nc.sync.dma_start(out=bqk_sb[:], in_=bq_d.ap())
        # preload the exp activation-table set (~2.7us) behind startup DMA
        dume = small.tile([1, 4], f32, tag="dume")
        nc.scalar.activation(dume[:], bqk_sb[0:1, 0:4], EXP)

        wv_sb = load_w(wv_d, "wv")
        load_x_chunk(xv, xvT_d, 0)
        wq_sb = load_w(wq_d, "wq")
        load_x_chunk(xq, xqT_d, 0)
        for c in range(1, NJ):
            load_x_chunk(xk, xkT_d, c)
            load_x_chunk(xv, xvT_d, c)
        wo_sb = consts.tile([P, 2 * D], MM_DT, tag="wo")  # 2 k-tiles [128, D]

        # persistent activations
        qT = [persist.tile([P, S], MM_DT, tag=f"qT{m}", name=f"qT{m}")
              for m in range(2)]
        kT = [persist.tile([P, S], MM_DT, tag=f"kT{m}", name=f"kT{m}")
              for m in range(2)]
        v_sb = persist.tile([P, HG * STILES * VBLK], MM_DT, tag="v")
        ctxT = [persist.tile([P, S], MM_DT, tag=f"ctxT{m}", name=f"ctxT{m}")
                for m in range(2)]
        v_view = v_sb.rearrange("p (h t c) -> p h t c", h=HG, t=STILES)

        # ones columns for the softmax-denominator rows of the PV matmuls
        nc.gpsimd.memset(v_view[:, :, :, HD], 1.0)

        def proj_qk_half(xc, w_sb, dst, bias_i, j, m):
            # dst[m][dq, j*CH:+CH] = (W[:, m-half].T @ x^T-chunk) + bias
            acc = ps_pr.tile([P, CH], f32, tag="pr", name="pr")
            for k in range(KT):
                nc.tensor.matmul(
                    acc[:], w_sb[:, k * DG + m * P:k * DG + (m + 1) * P],
                    xc[:, j, k, :],
                    start=(k == 0), stop=(k == KT - 1))
            nc.vector.tensor_add(
                dst[m][:, j * CH:(j + 1) * CH], acc[:],
                bqk_sb[:, 2 * bias_i + m:2 * bias_i + m + 1].broadcast_to(
                    [P, CH]))

        def proj_v(j):
            # v rows j*CH..: 4 s-subtiles of 128; heads land in v_view blocks
            for si in range(CH // P):
                st = j * (CH // P) + si
                acc = ps_pr.tile([P, DG], f32, tag="pr", name="pr")
                for k in range(KT):
                    nc.tensor.matmul(
                        acc[:], xv[:, j, k, si * P:(si + 1) * P],
                        wv_sb[:, k * DG:(k + 1) * DG],
                        start=(k == 0), stop=(k == KT - 1))
                nc.vector.tensor_copy(
                    v_view[:, :, st, 0:HD],
                    acc[:].rearrange("p (h c) -> p h c", h=HG))

        def scores_step(j, m, t):
            # Both heads of pair m into ONE psum tile: the PE runs the two
            # 64-row matmuls concurrently (row tiles (0,0)/(64,0)), and the
            # single tile gives both MMs one atomic WAR against exp(s-2).
            jc = slice(j * CH, (j + 1) * CH)
            sps = ps_s.tile([P, 2 * CH], f32, tag="s", name="s")
            for hh in range(2):
                lo, hi = hh * HD, (hh + 1) * HD
                nc.tensor.matmul(
                    sps[:, hh * CH:(hh + 1) * CH],
                    kT[m][lo:hi, t * P:(t + 1) * P],
                    qT[m][lo:hi, jc], start=True, stop=True)
            return sps

        def exp_step(sps):
            e = epool.tile([P, 2 * CH], MM_DT, tag="e", name="e")
            nc.scalar.activation(e[:], sps[:], EXP, scale=SCALE)
            return e

        def pv_step(accs, m, t, e):
            for hh in range(2):
                nc.tensor.matmul(
                    accs[hh][:], v_view[:, 2 * m + hh, t, :],
                    e[:, hh * CH:(hh + 1) * CH],
                    start=(t == 0), stop=(t == STILES - 1))

        def stage_accs(accs):
            # Copy psum ctx+denom to SBUF (bf16) right after the PV stop so
            # the psum banks free in ~0.8us.
            stg = []
            with nc.allow_low_precision(
                    reason="bf16 softmax ctx/denom: ~0.4% of a 2e-2 budget"):
                for hh in range(2):
                    s = bpool.tile([VBLK, CH], MM_DT, tag=f"stg{hh}",
                                   name="stg")
                    nc.vector.tensor_copy(s[:], accs[hh][:])
                    stg.append(s)
            return stg

        def recip_chain(denom_rows, dt):
            # 1/denom on 128 DVE lanes: reshape the [1,512] denominator rows
            # to [128,4] via sbuf-sbuf DMA (a 1-lane DVE reciprocal costs
            # ~3.3us; 128-lane costs ~70ns), then reshape back for the
            # partition_broadcast.  Reshape DMAs ride the idle GpSimd
            # (SWDGE) queue.
            dq = small.tile([P, 8], dt, tag="dq", name="dq")
            for hh in range(2):
                nc.gpsimd.dma_start(
                    dq[:, 4 * hh:4 * hh + 4],
                    denom_rows[hh].rearrange("o (p c) -> o p c", p=P))
            rq = small.tile([P, 8], dt, tag="rq", name="rq")
            with nc.allow_low_precision(
                    reason="bf16 softmax recip: ~0.4% of a 2e-2 budget"):
                nc.vector.reciprocal(rq[:], dq[:])
            rrs = []
            for hh in range(2):
                rr = bpool.tile([1, CH], dt, tag=f"rr{hh}", name="rr")
                nc.gpsimd.dma_start(
                    rr[:].rearrange("o (p c) -> o p c", p=P),
                    rq[:, 4 * hh:4 * hh + 4])
                rrs.append(rr)
            return rrs

        def apply_norm(j, m, stg, rrs, dt):
            # ctxT[m][hh-half, jc] = stg[0:HD] * pbcast(rr).  The broadcast
            # must land in a base-0 tile (pbcast to base 64 mis-broadcasts);
            # the DVE multiply handles the mixed in/out partition bases.
            jc = slice(j * CH, (j + 1) * CH)
            with nc.allow_low_precision(
                    reason="bf16 softmax normalize: ~0.4% of 2e-2 budget"):
                for hh in range(2):
                    bsb = bpool.tile([HD, CH], dt, tag="bsb", name="bsb")
                    nc.gpsimd.partition_broadcast(bsb[:], rrs[hh][:])
                    nc.vector.tensor_mul(
                        ctxT[m][hh * HD:(hh + 1) * HD, jc],
                        stg[hh][0:HD, :], bsb[:])

        def outproj_item(j, si, nh):
            # one [128, 512] block of chunk j's out-projection
            st = j * (CH // P) + si
            po = ps_pr.tile([P, CH], f32, tag="pr", name="po")
            for m in range(2):
                nc.tensor.matmul(
                    po[:], ctxT[m][:, st * P:(st + 1) * P],
                    wo_sb[:, m * D + nh * CH:m * D + (nh + 1) * CH],
                    start=(m == 0), stop=(m == 1))
            ostage = opool.tile([P, CH], MM_DT, tag="ostage", name="ostage")
            nc.vector.tensor_copy(ostage[:], po[:])
            nc.sync.dma_start(
                out_d[st * P:(st + 1) * P, nh * CH:(nh + 1) * CH],
                ostage[:])

        # ---- schedule: one flat software-pipelined stream over the 128
        # (j, m, t) steps.  Each step emits scores(s), pv(s-1), exp(s) -
        # in that order so the pipeline ops outrank same-step fillers in
        # the per-engine priority heaps - then the fillers (kv/q
        # projections, deferred x/wo DMAs, normalize and out-projection of
        # completed chunks).
        sections = [(j, m) for j in range(NJ) for m in range(2)]
        steps = [(k, j, m, t)
                 for k, (j, m) in enumerate(sections)
                 for t in range(STILES)]
        fillers = {}

        def add_filler(s, f):
            fillers.setdefault(s, []).append(f)

        def kv_filler(c):
            def f():
                for m in range(2):
                    proj_qk_half(xk, wk_sb, kT, 1, c, m)
            return f

        add_filler(1, kv_filler(1))
        add_filler(2, lambda: proj_v(1))
        add_filler(5, kv_filler(2))
        add_filler(6, lambda: proj_v(2))
        add_filler(9, kv_filler(3))
        add_filler(10, lambda: proj_v(3))
        add_filler(13, lambda: proj_qk_half(xq, wq_sb, qT, 0, 0, 1))
        add_filler(12, lambda: nc.sync.dma_start(out=wo_sb[:], in_=wo_d.ap()))
        for j in range(1, NJ):
            add_filler(16 * (j - 1) + 8,
                       lambda j=j: load_x_chunk(xq, xqT_d, j))
            add_filler(32 * j - 16,
                       lambda j=j: proj_qk_half(xq, wq_sb, qT, 0, j, 0))
            add_filler(32 * j + 2,
                       lambda j=j: proj_qk_half(xq, wq_sb, qT, 0, j, 1))

        def finish_section(k, s_now):
            # section k's accs are complete: stage out of psum, start the
            # reciprocal chain, and schedule its normalize + (for m=1) the
            # chunk's out-projection a few steps out.
            j, m = sections[k]
            accs = acc_state.pop(k)
            if s_now is None:
                # kernel tail: nothing needs the psum banks afterwards, so
                # stage only the denominator rows (GpSimd DMA can't read
                # PSUM) and normalize straight out of PSUM.
                dns = []
                for hh in range(2):
                    dn = bpool.tile([1, CH], f32, tag=f"dn{hh}", name="dn")
                    nc.vector.tensor_copy(dn[:], accs[hh][HD:HD + 1, :])
                    dns.append(dn)
                rrs = recip_chain(dns, f32)
                # keep the PE warm across the recip chain's ~4us so the
                # final out-projection runs at full clock (HAM)
                for w in range(12):
                    # tag "s": rotate the scores bufs - no extra psum banks
                    wps = ps_s.tile([P, CH], f32, tag="s", name="warm")
                    nc.tensor.matmul(wps[:], wk_sb[:, 0:P], xk[:, 0, 0, :],
                                     start=True, stop=True)
                apply_norm(j, m, [accs[hh] for hh in range(2)], rrs, f32)
                return
            stg = stage_accs(accs)
            rrs = recip_chain(
                [stg[hh][HD:HD + 1, :] for hh in range(2)], MM_DT)
            add_filler(s_now + 5,
                       lambda: apply_norm(j, m, stg, rrs, MM_DT))
            if m == 1:
                for i in range(8):
                    si, nh = divmod(i, 2)
                    add_filler(s_now + 6 + i,
                               lambda si=si, nh=nh: outproj_item(j, si, nh))

        # prologue: kv/q projections for chunk 0
        for m in range(2):
            proj_qk_half(xk, wk_sb, kT, 1, 0, m)
        proj_v(0)
        proj_qk_half(xq, wq_sb, qT, 0, 0, 0)

        acc_state = {}
        prev = None      # (k, m, t, e) of the previous step
        for s, (k, j, m, t) in enumerate(steps):
            sps = scores_step(j, m, t)
            if prev is not None:
                pk, pm, pt, pe = prev
                if pk not in acc_state:
                    acc_state[pk] = [
                        ps_acc.tile([VBLK, CH], f32, tag="acc", name=f"acc{i}")
                        for i in range(2)]
                pv_step(acc_state[pk], pm, pt, pe)
                if pt == STILES - 1:
                    finish_section(pk, s)
            e = exp_step(sps)
            for f in fillers.pop(s, ()):
                f()
            prev = (k, m, t, e)
        pk, pm, pt, pe = prev
        pv_step(acc_state[pk], pm, pt, pe)
        finish_section(pk, None)
        for si in range(CH // P):
            for nh in range(2):
                outproj_item(NJ - 1, si, nh)
        for f in sorted(fillers):
            for g in fillers[f]:
                g()


_NC_CACHE = {}


def _get_program():
    if "nc" not in _NC_CACHE:
        _NC_CACHE["nc"] = _build_program()
    return _NC_CACHE["nc"]


def _make_in_maps(inputs):
    query = np.asarray(inputs["query"], dtype=np.float32)
    key = np.asarray(inputs["key"], dtype=np.float32)
    value = np.asarray(inputs["value"], dtype=np.float32)
    Wq = np.asarray(inputs["Wq"], dtype=np.float32)
    Wk = np.asarray(inputs["Wk"], dtype=np.float32)
    Wv = np.asarray(inputs["Wv"], dtype=np.float32)
    Wo = np.asarray(inputs["Wo"], dtype=np.float32)
    bq = np.asarray(inputs["bq"], dtype=np.float32)
    bk = np.asarray(inputs["bk"], dtype=np.float32)

    # pre-tile into the device SBUF layouts (partition-major; x chunk-major)
    def tile_x(xTb):  # [D, S] -> [P, NJ*KT*CH]
        return np.ascontiguousarray(
            xTb.reshape(KT, P, NJ, CH).transpose(1, 2, 0, 3).reshape(
                P, NJ * KT * CH))

    def tile_w(w):  # [D, cols] -> [P, KT*cols]
        n = w.shape[1]
        return np.ascontiguousarray(
            w.reshape(KT, P, n).transpose(1, 0, 2).reshape(P, KT * n))

    xT = {}
    for b in range(B):
        xT[("q", b)] = tile_x(query[b].T.astype(NP_MM))
        xT[("k", b)] = tile_x(key[b].T.astype(NP_MM))
        xT[("v", b)] = tile_x(value[b].T.astype(NP_MM))

    in_maps = []
    for c in range(NCORE):
        b, g = divmod(c, G)
        cols = slice(g * DG, (g + 1) * DG)
        wo_g = Wo[cols, :].astype(NP_MM)  # [DG, D] = [(2 kt, P), D]
        in_maps.append({
            "xqT": xT[("q", b)],
            "xkT": xT[("k", b)],
            "xvT": xT[("v", b)],
            "wq": tile_w(Wq[:, cols].astype(NP_MM)),
            "wk": tile_w(Wk[:, cols].astype(NP_MM)),
            "wv": tile_w(Wv[:, cols].astype(NP_MM)),
            "wo": np.ascontiguousarray(
                wo_g.reshape(2, P, D).transpose(1, 0, 2).reshape(P, 2 * D)),
            "bqk": np.ascontiguousarray(
                np.stack([bq[cols], bk[cols]]).reshape(2, 2, P).transpose(
                    2, 0, 1).reshape(P, 4)),
        })
    return in_maps


def kernel(query, key, value, Wq, bq, Wk, bk, Wv, bv, Wo, bo):
    bv = np.asarray(bv, dtype=np.float32)
    bo = np.asarray(bo, dtype=np.float32)
    Wo = np.asarray(Wo, dtype=np.float32)

    nc = _get_program()
    in_maps = _make_in_maps({
        "query": query, "key": key, "value": value, "Wq": Wq, "Wk": Wk,
        "Wv": Wv, "Wo": Wo, "bq": bq, "bk": bk,
    })

    res = run_bass_kernel_spmd(nc, in_maps, list(range(NCORE)))

    # unshard: sum the 4 head-group partials per batch (bf16 -> f32); add
    # the linear bias correction (bv and bo commute through attn/out_proj).
    corr = bv @ Wo + bo
    out = np.empty((B, S, D), dtype=np.float32)
    for b in range(B):
        acc = res.results[4 * b]["out"].astype(np.float32)
        for g in range(1, G):
            acc += res.results[4 * b + g]["out"].astype(np.float32)
        out[b] = acc + corr
    return out
